# revision 11
# baseline (speedup 1.0000x reference)
"""Trainium2 Bass kernel for nn_EquivariantLayer (GNN message passing).

Computation (see module docstring in the problem's reference):
    lw  = weights[:, ROT] reshaped            (1280, 512), ROT closed-form
    msg = conn_vals[:, None] * x[conn_cols]   (NNZ, 16)
    agg = segment_sum(msg, conn_rows)         (N*80, 16)
    out = agg.reshape(N, 1280) @ lw           (N, 512)

Distribution: edges sharded by destination vertex across 8 NeuronCores
(2500 vertices / 200k destination rows each); x is replicated.

Per-core device pipeline:
  * x.T replicated 8x lives in SBUF [128, 20000]; the per-edge gather
    x[cols] runs on GPSIMD (`ap_gather`, 8 Q7 cores each serving its own
    16-partition channel group with its own slot list).
  * Edge slots are grouped in chunks of 128, one destination vertex per
    chunk ("template" of chunk counts shared by all cores so a single SPMD
    program serves all 8).  Gather output [16ch x slots] is flipped to
    [slots x ch] with PE transposes (8 chunks per 128x128 transpose), and
    scaled by conn_vals on the way out of PSUM.
  * Scatter within a vertex: a 0/1 mask [128 slots x 80 bins] built with one
    DVE is_equal against an iota row; matmul msg.T @ mask accumulates the
    vertex's aggregate [16 x 80] in PSUM over its chunks.
  * Aggregates are restaged (static DMAs) into h^T [128=(b%8)*16+c x v*10]
    and the dense matmul h @ lw runs as 10 accumulating 128x128x512 fp32
    matmuls per 128-vertex block.
"""

import numpy as np

# ---------------------------------------------------------------- constants
P_, T_ = 5, 16
B = 80                  # bins per vertex
C_IN = 16
N_VERTS = 20000
K = 1280
N_CORES = 8
NV = N_VERTS // N_CORES  # vertices per core
VBLK = 128               # vertices per processing block
CH = 128                 # slots per chunk
O = 512                  # output features


# ------------------------------------------------------------- host prep
def _build_template(rows):
    """Chunks-per-vertex template shared by all cores (max over cores)."""
    v = rows // B
    d = v // NV
    i = v % NV
    deg = np.zeros((N_CORES, NV), np.int64)
    np.add.at(deg, (d, i), 1)
    T = np.maximum(1, -(-np.maximum(deg, 1) // CH)).max(axis=0)
    nvb = -(-NV // VBLK)
    vb_slices = [slice(w * VBLK, min((w + 1) * VBLK, NV)) for w in range(nvb)]
    for sl in vb_slices:
        c = int(T[sl].sum())
        T[sl.stop - 1] += (-c) % 16   # chunk count per vblock multiple of 16
    return T, vb_slices


def _build_core_arrays(T, vb_slices, rows_d, cols_d, vals_d, vbase):
    """Per-core [128, COLS] arrays: ap_gather idx (int16), vals, row_local."""
    v_local = rows_d // B - vbase
    b = (rows_d % B).astype(np.float32)
    order = np.argsort(v_local, kind="stable")
    v_s = v_local[order]
    b_s = b[order]
    c_s = cols_d[order].astype(np.int16)
    w_s = vals_d[order].astype(np.float32)
    starts = np.searchsorted(v_s, np.arange(NV + 1))

    idx_parts, vals_parts, rl_parts = [], [], []
    for sl in vb_slices:
        Cw = int(T[sl].sum())
        n_tp = Cw // 8
        nslots = Cw * CH
        slot_col = np.zeros(nslots, np.int16)
        slot_val = np.zeros(nslots, np.float32)
        slot_rl = np.zeros(nslots, np.float32)
        cstart = 0
        for i in range(sl.start, sl.stop):
            e0, e1 = int(starts[i]), int(starts[i + 1])
            ne = e1 - e0
            s0 = cstart * CH
            slot_col[s0:s0 + ne] = c_s[e0:e1]
            slot_val[s0:s0 + ne] = w_s[e0:e1]
            slot_rl[s0:s0 + ne] = b_s[e0:e1]
            cstart += int(T[i])
        sc = np.arange(nslots) // CH
        ss = np.arange(nslots) % CH
        g = sc % 8
        p = sc // 8
        q = p * CH + ss
        idx = np.zeros((128, n_tp * 8), np.int16)
        idx[16 * g + q % 16, q // 16] = slot_col
        vals = np.zeros((128, n_tp * 8), np.float32)   # [s, (t g)]
        rl = np.zeros((128, n_tp * 8), np.float32)
        vals[ss, p * 8 + g] = slot_val
        rl[ss, p * 8 + g] = slot_rl
        idx_parts.append(idx)
        vals_parts.append(vals)
        rl_parts.append(rl)
    return (np.concatenate(idx_parts, 1), np.concatenate(vals_parts, 1),
            np.concatenate(rl_parts, 1))


def _lw_from_weights(weights):
    """layer_weights (K, 512) from weights via the closed-form ROT table."""
    k = np.arange(K)
    ci, u = k // B, k % B
    t, p = u // P_, u % P_
    j = np.arange(16)
    rot = ci[:, None] * B + ((t[:, None] + j) % T_) * P_ + p[:, None]
    lw = weights[:, rot]                     # (32, K, 16)
    return np.transpose(lw, (1, 0, 2)).reshape(K, 32 * 16)


# ------------------------------------------------------------ bass program
def _build_program(T, vb_slices, cols_total):
    from concourse import bacc, mybir, tile
    import concourse.bass as bass

    nc = bacc.Bacc("TRN2", target_bir_lowering=False, debug=False,
                   num_devices=N_CORES)
    f32 = mybir.dt.float32
    xT_in = nc.dram_tensor("xT", [128, N_VERTS], f32, kind="ExternalInput")
    # layer_weights replicated per the sharding contract; host pre-arranges
    # rows k = k_hi*128 + p into [p, (k_hi, o)]
    lw_in = nc.dram_tensor("lw", [128, 10 * O], f32, kind="ExternalInput")
    idx_in = nc.dram_tensor("idx16", [128, cols_total], mybir.dt.int16,
                            kind="ExternalInput")
    vals_in = nc.dram_tensor("vals", [128, cols_total], f32,
                             kind="ExternalInput")
    rl_in = nc.dram_tensor("rl", [128, cols_total], f32, kind="ExternalInput")
    iota_in = nc.dram_tensor("iota", [128, 8 * B], f32, kind="ExternalInput")
    id_in = nc.dram_tensor("ident", [128, 128], f32, kind="ExternalInput")
    y_out = nc.dram_tensor("y", [NV, O], f32, kind="ExternalOutput")

    with tile.TileContext(nc) as tc:
        with (
            tc.tile_pool(name="persist", bufs=1) as pp,
            tc.tile_pool(name="io", bufs=2) as iop,
            tc.tile_pool(name="raw", bufs=2) as rawp,
            tc.tile_pool(name="msg", bufs=4) as msgp,
            tc.tile_pool(name="mask", bufs=4) as maskp,
            tc.tile_pool(name="stage", bufs=2) as stp,
            tc.tile_pool(name="ht", bufs=2) as htp,
            tc.tile_pool(name="outp", bufs=2) as outp,
            tc.tile_pool(name="tpsum", bufs=2, space="PSUM") as tps,
            tc.tile_pool(name="vpsum", bufs=4, space="PSUM") as vps,
            tc.tile_pool(name="opsum", bufs=2, space="PSUM") as ops,
        ):
            # ---- layer_weights (pre-arranged on host, replicated)
            lw_sb = pp.tile([128, 10 * O], f32)
            nc.sync.dma_start(lw_sb[:], lw_in[:])

            # ---- persistent tiles
            xT = pp.tile([128, N_VERTS], f32)
            nc.sync.dma_start(xT[:], xT_in[:])
            iota = pp.tile([128, 8 * B], f32)
            nc.sync.dma_start(iota[:], iota_in[:])
            ident = pp.tile([128, 128], f32)
            nc.sync.dma_start(ident[:], id_in[:])

            col_off = 0
            for sl in vb_slices:
                nv = sl.stop - sl.start
                Tw = [int(T[i]) for i in range(sl.start, sl.stop)]
                Cw = sum(Tw)
                n_tp = Cw // 8
                ncols = n_tp * 8
                chunk_vertex = []   # local vertex index per global chunk
                for li, ti in enumerate(Tw):
                    chunk_vertex += [li] * ti

                idx_t = iop.tile([128, ncols], mybir.dt.int16, tag="idx")
                nc.sync.dma_start(idx_t[:], idx_in[:, col_off:col_off + ncols])
                vals_t = iop.tile([128, ncols], f32, tag="vals")
                nc.sync.dma_start(vals_t[:], vals_in[:, col_off:col_off + ncols])
                rl_t = iop.tile([128, ncols], f32, tag="rl")
                nc.sync.dma_start(rl_t[:], rl_in[:, col_off:col_off + ncols])

                # gather in halves for overlap
                half = n_tp // 2
                raws = []
                for h, (t0, t1) in enumerate(((0, half), (half, n_tp))):
                    ntp_h = t1 - t0
                    if ntp_h == 0:
                        continue
                    raw = rawp.tile([128, ntp_h * CH], f32, tag="raw")
                    nc.gpsimd.ap_gather(
                        raw[:], xT[:], idx_t[:, t0 * 8:t1 * 8],
                        channels=128, num_elems=N_VERTS, d=1,
                        num_idxs=ntp_h * CH)
                    raws.append((t0, raw))

                # per-vertex psum state
                vcur = {}
                done = 0
                stage = None
                svert0 = 0

                def flush_stage(upto):
                    nonlocal stage, svert0
                    if stage is None:
                        return
                    nv2 = upto - svert0
                    sr = stage[:].rearrange("c (v bh bl) -> c v bh bl",
                                            bh=10, bl=8)
                    hr = h_T[:].rearrange("p (v bh) -> p v bh", bh=10)
                    for bl in range(8):
                        nc.sync.dma_start(
                            hr[bl * 16:(bl + 1) * 16,
                               svert0:svert0 + nv2, :],
                            sr[:, :nv2, :, bl])
                    stage = None

                h_T = htp.tile([128, VBLK * 10], f32, tag="ht")
                ci = 0
                for t0, raw in raws:
                    ntp_h = raw.shape[1] // CH
                    for tt in range(ntp_h):
                        t = t0 + tt
                        ps = tps.tile([128, 128], f32, tag="tp")
                        nc.tensor.transpose(ps[:], raw[:, tt * CH:(tt + 1) * CH],
                                            ident[:])
                        msg = msgp.tile([128, 128], f32, tag="msg")
                        v_b = vals_t[:, t * 8:(t + 1) * 8].unsqueeze(2) \
                            .broadcast_to([128, 8, 16])
                        nc.vector.tensor_tensor(
                            out=msg[:].rearrange("s (g c) -> s g c", g=8),
                            in0=ps[:].rearrange("s (g c) -> s g c", g=8),
                            in1=v_b, op=mybir.AluOpType.mult)
                        mask = maskp.tile([128, 8 * B], f32, tag="mask")
                        r_b = rl_t[:, t * 8:(t + 1) * 8].unsqueeze(2) \
                            .broadcast_to([128, 8, B])
                        nc.vector.tensor_tensor(
                            out=mask[:].rearrange("s (g b) -> s g b", g=8),
                            in0=iota[:].rearrange("s (g b) -> s g b", g=8),
                            in1=r_b, op=mybir.AluOpType.is_equal)
                        for g in range(8):
                            c = t * 8 + g
                            vi = chunk_vertex[c]
                            first = vi not in vcur
                            if first:
                                vcur[vi] = vps.tile([16, B], f32, tag="vp", name=f"vp{t}_{g}")
                            last = (c + 1 == Cw) or (chunk_vertex[c + 1] != vi)
                            nc.tensor.matmul(
                                vcur[vi][:], lhsT=msg[:, g * 16:(g + 1) * 16],
                                rhs=mask[:, g * B:(g + 1) * B],
                                start=first, stop=last)
                            if last:
                                if stage is None:
                                    stage = stp.tile([16, 32 * B], f32, tag="stage", name=f"stage{vi}")
                                    svert0 = vi
                                nc.vector.tensor_copy(
                                    stage[:, (vi - svert0) * B:
                                          (vi - svert0 + 1) * B],
                                    vcur.pop(vi)[:])
                                done += 1
                                if vi - svert0 == 31:
                                    flush_stage(vi + 1)
                        ci += 8
                flush_stage(nv)

                opsum = ops.tile([128, O], f32, tag="op")
                hr2 = h_T[:].rearrange("p (v bh) -> p v bh", bh=10)
                for k_hi in range(10):
                    nc.tensor.matmul(
                        opsum[0:nv, :], lhsT=hr2[:, 0:nv, k_hi],
                        rhs=lw_sb[:, k_hi * O:(k_hi + 1) * O],
                        start=(k_hi == 0), stop=(k_hi == 9))
                out_sb = outp.tile([128, O], f32, tag="osb")
                nc.vector.tensor_copy(out_sb[0:nv, :], opsum[0:nv, :])
                nc.sync.dma_start(y_out[sl.start:sl.stop, :], out_sb[0:nv, :])
                col_off += ncols
    nc.compile()
    return nc


# ---------------------------------------------------------------- kernel
def kernel(x, weights, conn_vals, conn_rows, conn_cols):
    import sys
    for p in ("/opt/trn_rl_repo",):
        if p not in sys.path:
            sys.path.append(p)
    from concourse.bass_utils import run_bass_kernel_spmd

    x = np.asarray(x)
    weights = np.asarray(weights)
    conn_vals = np.asarray(conn_vals, dtype=np.float32)
    rows = np.asarray(conn_rows).astype(np.int64)
    cols = np.asarray(conn_cols).astype(np.int64)

    T, vb_slices = _build_template(rows)
    cols_total = int(T.sum())  # one idx/vals/rl column per chunk slot-16-group

    nc = _build_program(T, vb_slices, cols_total)

    xT = np.ascontiguousarray(np.tile(x.T.astype(np.float32), (8, 1)))
    iota = np.ascontiguousarray(
        np.tile(np.arange(B, dtype=np.float32), (128, 8)))
    ident = np.eye(128, dtype=np.float32)
    lw_arr = np.ascontiguousarray(
        _lw_from_weights(weights.astype(np.float32))
        .reshape(10, 128, O).transpose(1, 0, 2).reshape(128, 10 * O))

    d_of = rows // (NV * B)
    in_maps = []
    for d in range(N_CORES):
        m = d_of == d
        idx16, vals, rl = _build_core_arrays(
            T, vb_slices, rows[m], cols[m], conn_vals[m], d * NV)
        assert idx16.shape[1] == cols_total, (idx16.shape, cols_total)
        in_maps.append({
            "xT": xT, "lw": lw_arr,
            "idx16": idx16, "vals": vals, "rl": rl,
            "iota": iota, "ident": ident,
        })

    res = run_bass_kernel_spmd(nc, in_maps, core_ids=list(range(N_CORES)),
                               trace=bool(globals().get("TRACE", False)))
    global LAST_EXEC_NS, LAST_RESULTS
    LAST_EXEC_NS = res.exec_time_ns
    LAST_RESULTS = res
    out = np.concatenate([res.results[d]["y"] for d in range(N_CORES)], axis=0)
    return out.astype(np.float32)


if __name__ == "__main__":
    pass


# revision 14
# speedup vs baseline: 1.0696x; 1.0696x over previous
"""Trainium2 Bass kernel for nn_EquivariantLayer (GNN message passing).

Computation (see module docstring in the problem's reference):
    lw  = weights[:, ROT] reshaped            (1280, 512), ROT closed-form
    msg = conn_vals[:, None] * x[conn_cols]   (NNZ, 16)
    agg = segment_sum(msg, conn_rows)         (N*80, 16)
    out = agg.reshape(N, 1280) @ lw           (N, 512)

Distribution: edges sharded by destination vertex across 8 NeuronCores
(2500 vertices / 200k destination rows each); x is replicated.

Per-core device pipeline:
  * x.T replicated 8x lives in SBUF [128, 20000]; the per-edge gather
    x[cols] runs on GPSIMD (`ap_gather`, 8 Q7 cores each serving its own
    16-partition channel group with its own slot list).
  * Edge slots are grouped in chunks of 128, one destination vertex per
    chunk ("template" of chunk counts shared by all cores so a single SPMD
    program serves all 8).  Gather output [16ch x slots] is flipped to
    [slots x ch] with PE transposes (8 chunks per 128x128 transpose), and
    scaled by conn_vals on the way out of PSUM.
  * Scatter within a vertex: a 0/1 mask [128 slots x 80 bins] built with one
    DVE is_equal against an iota row; matmul msg.T @ mask accumulates the
    vertex's aggregate [16 x 80] in PSUM over its chunks.
  * Aggregates are restaged (static DMAs) into h^T [128=(b%8)*16+c x v*10]
    and the dense matmul h @ lw runs as 10 accumulating 128x128x512 fp32
    matmuls per 128-vertex block.
"""

import numpy as np

# ---------------------------------------------------------------- constants
P_, T_ = 5, 16
B = 80                  # bins per vertex
C_IN = 16
N_VERTS = 20000
K = 1280
N_CORES = 8
NV = N_VERTS // N_CORES  # vertices per core
VBLK = 128               # vertices per processing block
CH = 128                 # slots per chunk
O = 512                  # output features


# ------------------------------------------------------------- host prep
def _build_template(rows):
    """Chunks-per-vertex template shared by all cores (max over cores)."""
    v = rows // B
    d = v // NV
    i = v % NV
    deg = np.zeros((N_CORES, NV), np.int64)
    np.add.at(deg, (d, i), 1)
    T = np.maximum(1, -(-np.maximum(deg, 1) // CH)).max(axis=0)
    nvb = -(-NV // VBLK)
    vb_slices = [slice(w * VBLK, min((w + 1) * VBLK, NV)) for w in range(nvb)]
    for sl in vb_slices:
        c = int(T[sl].sum())
        T[sl.stop - 1] += (-c) % 16   # chunk count per vblock multiple of 16
    return T, vb_slices


def _build_core_arrays(T, vb_slices, rows_d, cols_d, vals_d, vbase):
    """Per-core [128, COLS] arrays: ap_gather idx (int16), vals, row_local."""
    v_local = rows_d // B - vbase
    b = (rows_d % B).astype(np.float32)
    order = np.argsort(v_local, kind="stable")
    v_s = v_local[order]
    b_s = b[order]
    c_s = cols_d[order].astype(np.int16)
    w_s = vals_d[order].astype(np.float32)
    starts = np.searchsorted(v_s, np.arange(NV + 1))

    idx_parts, vals_parts, rl_parts = [], [], []
    for sl in vb_slices:
        Cw = int(T[sl].sum())
        n_tp = Cw // 8
        nslots = Cw * CH
        slot_col = np.zeros(nslots, np.int16)
        slot_val = np.zeros(nslots, np.float32)
        slot_rl = np.zeros(nslots, np.float32)
        cstart = 0
        for i in range(sl.start, sl.stop):
            e0, e1 = int(starts[i]), int(starts[i + 1])
            ne = e1 - e0
            s0 = cstart * CH
            slot_col[s0:s0 + ne] = c_s[e0:e1]
            slot_val[s0:s0 + ne] = w_s[e0:e1]
            slot_rl[s0:s0 + ne] = b_s[e0:e1]
            cstart += int(T[i])
        sc = np.arange(nslots) // CH
        ss = np.arange(nslots) % CH
        g = sc % 8
        p = sc // 8
        q = p * CH + ss
        idx = np.zeros((128, n_tp * 8), np.int16)
        idx[16 * g + q % 16, q // 16] = slot_col
        vals = np.zeros((128, n_tp * 8), np.float32)   # [s, (t g)]
        rl = np.zeros((128, n_tp * 8), np.float32)
        vals[ss, p * 8 + g] = slot_val
        rl[ss, p * 8 + g] = slot_rl
        idx_parts.append(idx)
        vals_parts.append(vals)
        rl_parts.append(rl)
    return (np.concatenate(idx_parts, 1), np.concatenate(vals_parts, 1),
            np.concatenate(rl_parts, 1))


def _lw_from_weights(weights):
    """layer_weights (K, 512) from weights via the closed-form ROT table."""
    k = np.arange(K)
    ci, u = k // B, k % B
    t, p = u // P_, u % P_
    j = np.arange(16)
    rot = ci[:, None] * B + ((t[:, None] + j) % T_) * P_ + p[:, None]
    lw = weights[:, rot]                     # (32, K, 16)
    return np.transpose(lw, (1, 0, 2)).reshape(K, 32 * 16)


# ------------------------------------------------------------ bass program
def _build_program(T, vb_slices, cols_total):
    from concourse import bacc, mybir, tile
    import concourse.bass as bass

    nc = bacc.Bacc("TRN2", target_bir_lowering=False, debug=False,
                   num_devices=N_CORES)
    f32 = mybir.dt.float32
    xT_in = nc.dram_tensor("xT", [128, N_VERTS], f32, kind="ExternalInput")
    # layer_weights replicated per the sharding contract; host pre-arranges
    # rows k = k_hi*128 + p into [p, (k_hi, o)]
    lw_in = nc.dram_tensor("lw", [128, 10 * O], f32, kind="ExternalInput")
    idx_in = nc.dram_tensor("idx16", [128, cols_total], mybir.dt.int16,
                            kind="ExternalInput")
    vals_in = nc.dram_tensor("vals", [128, cols_total], f32,
                             kind="ExternalInput")
    rl_in = nc.dram_tensor("rl", [128, cols_total], f32, kind="ExternalInput")
    iota_in = nc.dram_tensor("iota", [128, 8 * B], f32, kind="ExternalInput")
    id_in = nc.dram_tensor("ident", [128, 128], f32, kind="ExternalInput")
    y_out = nc.dram_tensor("y", [NV, O], f32, kind="ExternalOutput")

    with tile.TileContext(nc) as tc:
        with (
            tc.tile_pool(name="persist", bufs=1) as pp,
            tc.tile_pool(name="io", bufs=2) as iop,
            tc.tile_pool(name="raw", bufs=2) as rawp,
            tc.tile_pool(name="msg", bufs=4) as msgp,
            tc.tile_pool(name="mask", bufs=4) as maskp,
            tc.tile_pool(name="stage", bufs=2) as stp,
            tc.tile_pool(name="ht", bufs=2) as htp,
            tc.tile_pool(name="outp", bufs=2) as outp,
            tc.tile_pool(name="tpsum", bufs=2, space="PSUM") as tps,
            tc.tile_pool(name="vpsum", bufs=4, space="PSUM") as vps,
            tc.tile_pool(name="opsum", bufs=2, space="PSUM") as ops,
        ):
            # ---- layer_weights (pre-arranged on host, replicated)
            lw_sb = pp.tile([128, 10 * O], f32)
            nc.sync.dma_start(lw_sb[:], lw_in[:])

            # ---- persistent tiles
            xT = pp.tile([128, N_VERTS], f32)
            nc.sync.dma_start(xT[:], xT_in[:])
            iota = pp.tile([128, 8 * B], f32)
            nc.sync.dma_start(iota[:], iota_in[:])
            ident = pp.tile([128, 128], f32)
            nc.sync.dma_start(ident[:], id_in[:])

            col_off = 0
            for sl in vb_slices:
                nv = sl.stop - sl.start
                Tw = [int(T[i]) for i in range(sl.start, sl.stop)]
                Cw = sum(Tw)
                n_tp = Cw // 8
                ncols = n_tp * 8
                chunk_vertex = []   # local vertex index per global chunk
                for li, ti in enumerate(Tw):
                    chunk_vertex += [li] * ti

                idx_t = iop.tile([128, ncols], mybir.dt.int16, tag="idx")
                nc.sync.dma_start(idx_t[:], idx_in[:, col_off:col_off + ncols])
                vals_t = iop.tile([128, ncols], f32, tag="vals")
                nc.sync.dma_start(vals_t[:], vals_in[:, col_off:col_off + ncols])
                rl_t = iop.tile([128, ncols], f32, tag="rl")
                nc.sync.dma_start(rl_t[:], rl_in[:, col_off:col_off + ncols])

                # gather in halves for overlap
                half = n_tp // 2
                raws = []
                for h, (t0, t1) in enumerate(((0, half), (half, n_tp))):
                    ntp_h = t1 - t0
                    if ntp_h == 0:
                        continue
                    raw = rawp.tile([128, ntp_h * CH], f32, tag="raw")
                    nc.gpsimd.ap_gather(
                        raw[:], xT[:], idx_t[:, t0 * 8:t1 * 8],
                        channels=128, num_elems=N_VERTS, d=1,
                        num_idxs=ntp_h * CH)
                    raws.append((t0, raw))

                # per-vertex psum state
                vcur = {}
                done = 0
                stage = None
                svert0 = 0

                def flush_stage(upto):
                    # staging free layout: (bl, bh, v32); h_T free: (bh, v)
                    nonlocal stage, svert0
                    if stage is None:
                        return
                    nv2 = upto - svert0
                    sr = stage[:].rearrange("c (bl bh v) -> c bl bh v",
                                            bl=8, bh=10)
                    hr = h_T[:].rearrange("p (bh v) -> p bh v", bh=10)
                    for bl in range(8):
                        nc.sync.dma_start(
                            hr[bl * 16:(bl + 1) * 16, :,
                               svert0:svert0 + nv2],
                            sr[:, bl, :, :nv2])
                    stage = None

                h_T = htp.tile([128, VBLK * 10], f32, tag="ht")
                ci = 0
                for t0, raw in raws:
                    ntp_h = raw.shape[1] // CH
                    for tt in range(ntp_h):
                        t = t0 + tt
                        ps = tps.tile([128, 128], f32, tag="tp")
                        nc.tensor.transpose(ps[:], raw[:, tt * CH:(tt + 1) * CH],
                                            ident[:])
                        msg = msgp.tile([128, 128], f32, tag="msg")
                        v_b = vals_t[:, t * 8:(t + 1) * 8].unsqueeze(2) \
                            .broadcast_to([128, 8, 16])
                        nc.vector.tensor_tensor(
                            out=msg[:].rearrange("s (g c) -> s g c", g=8),
                            in0=ps[:].rearrange("s (g c) -> s g c", g=8),
                            in1=v_b, op=mybir.AluOpType.mult)
                        mask = maskp.tile([128, 8 * B], f32, tag="mask")
                        r_b = rl_t[:, t * 8:(t + 1) * 8].unsqueeze(2) \
                            .broadcast_to([128, 8, B])
                        nc.vector.tensor_tensor(
                            out=mask[:].rearrange("s (g b) -> s g b", g=8),
                            in0=iota[:].rearrange("s (g b) -> s g b", g=8),
                            in1=r_b, op=mybir.AluOpType.is_equal)
                        for g in range(8):
                            c = t * 8 + g
                            vi = chunk_vertex[c]
                            first = vi not in vcur
                            if first:
                                vcur[vi] = vps.tile([16, B], f32, tag="vp", name=f"vp{t}_{g}")
                            last = (c + 1 == Cw) or (chunk_vertex[c + 1] != vi)
                            nc.tensor.matmul(
                                vcur[vi][:], lhsT=msg[:, g * 16:(g + 1) * 16],
                                rhs=mask[:, g * B:(g + 1) * B],
                                start=first, stop=last)
                            if last:
                                if stage is None:
                                    stage = stp.tile([16, 32 * B], f32, tag="stage", name=f"stage{vi}")
                                    svert0 = vi
                                so = stage[:].rearrange(
                                    "c (bl bh v) -> c bl bh v", bl=8, bh=10)
                                nc.scalar.copy(
                                    so[:, :, :, vi - svert0],
                                    vcur.pop(vi)[:].rearrange(
                                        "c (bh bl) -> c bl bh", bl=8))
                                done += 1
                                if vi - svert0 == 31:
                                    flush_stage(vi + 1)
                        ci += 8
                flush_stage(nv)

                opsum = ops.tile([128, O], f32, tag="op")
                for k_hi in range(10):
                    nc.tensor.matmul(
                        opsum[0:nv, :],
                        lhsT=h_T[:, k_hi * VBLK:k_hi * VBLK + nv],
                        rhs=lw_sb[:, k_hi * O:(k_hi + 1) * O],
                        start=(k_hi == 0), stop=(k_hi == 9))
                out_sb = outp.tile([128, O], f32, tag="osb")
                nc.vector.tensor_copy(out_sb[0:nv, :], opsum[0:nv, :])
                nc.sync.dma_start(y_out[sl.start:sl.stop, :], out_sb[0:nv, :])
                col_off += ncols
    nc.compile()
    return nc


# ---------------------------------------------------------------- kernel
def kernel(x, weights, conn_vals, conn_rows, conn_cols):
    import sys
    for p in ("/opt/trn_rl_repo",):
        if p not in sys.path:
            sys.path.append(p)
    from concourse.bass_utils import run_bass_kernel_spmd

    x = np.asarray(x)
    weights = np.asarray(weights)
    conn_vals = np.asarray(conn_vals, dtype=np.float32)
    rows = np.asarray(conn_rows).astype(np.int64)
    cols = np.asarray(conn_cols).astype(np.int64)

    T, vb_slices = _build_template(rows)
    cols_total = int(T.sum())  # one idx/vals/rl column per chunk slot-16-group

    nc = _build_program(T, vb_slices, cols_total)

    xT = np.ascontiguousarray(np.tile(x.T.astype(np.float32), (8, 1)))
    iota = np.ascontiguousarray(
        np.tile(np.arange(B, dtype=np.float32), (128, 8)))
    ident = np.eye(128, dtype=np.float32)
    lw_arr = np.ascontiguousarray(
        _lw_from_weights(weights.astype(np.float32))
        .reshape(10, 128, O).transpose(1, 0, 2).reshape(128, 10 * O))

    d_of = rows // (NV * B)
    in_maps = []
    for d in range(N_CORES):
        m = d_of == d
        idx16, vals, rl = _build_core_arrays(
            T, vb_slices, rows[m], cols[m], conn_vals[m], d * NV)
        assert idx16.shape[1] == cols_total, (idx16.shape, cols_total)
        in_maps.append({
            "xT": xT, "lw": lw_arr,
            "idx16": idx16, "vals": vals, "rl": rl,
            "iota": iota, "ident": ident,
        })

    res = run_bass_kernel_spmd(nc, in_maps, core_ids=list(range(N_CORES)),
                               trace=bool(globals().get("TRACE", False)))
    global LAST_EXEC_NS, LAST_RESULTS
    LAST_EXEC_NS = res.exec_time_ns
    LAST_RESULTS = res
    out = np.concatenate([res.results[d]["y"] for d in range(N_CORES)], axis=0)
    return out.astype(np.float32)


if __name__ == "__main__":
    pass
